# revision 12
# baseline (speedup 1.0000x reference)
"""BP-MLL loss kernel for Trainium2 (8 NeuronCores, data-parallel over batch).

Math: for each sample b with scores o and binary labels y,
  pair_sums[b] = sum_{i in pos, j in neg} exp(o_j - o_i)
               = (sum_{j in neg} exp(o_j)) * (sum_{i in pos} exp(-o_i))
  y_norm[b]    = n_pos * (C - n_pos)
  loss         = sum_b pair_sums[b] / y_norm[b] / B

Since labels are 0/1, the masks fold into the exp arguments on the host:
  w = where(y==0,  x, -BIG)   ->  exp(w) = (1-y)*exp(x)   (underflows to 0)
  v = where(y==1, -x, -BIG)   ->  exp(v) =     y*exp(-x)
Each core gets 4 samples packed as one [128, 128] f32 buffer (cols 0:64 = w,
cols 64:128 = v; sample b owns partitions 32b..32b+31). The device runs a
single Exp activation and a single 2-group free-axis reduce, emitting
[128, 2] per-partition partial sums; the host finishes the tiny segmented
reduction (n_pos comes straight from `target` on the host).
"""

import sys

for _p in ("/opt/trn_rl_repo", "/root/.axon_site/_ro/trn_rl_repo"):
    if _p not in sys.path:
        sys.path.insert(0, _p)

import numpy as np

import concourse.bass as bass
import concourse.mybir as mybir
from concourse.bass_utils import run_bass_kernel_spmd

B, C = 32, 2048
N_CORES = 8
BPC = B // N_CORES            # samples per core (4)
P = 128                       # SBUF partitions
F = BPC * C // P              # free elems per partition (64)
PPS = P // BPC                # partitions per sample (32)
BIG = np.float32(30000.0)     # exp(x - BIG) underflows to +0 for any |x| here

_NC_CACHE = {}
# Extra kwargs for run_bass_kernel_spmd (e.g. trace=True from a test harness).
_RUN_KWARGS = {}


def _build_bass():
    nc = bass.Bass("TRN2", enable_partition_id=False)
    fp32 = mybir.dt.float32
    x_d = nc.declare_dram_parameter("x", [P, 2 * F], fp32, isOutput=False)
    o_d = nc.declare_dram_parameter("out", [P, 2], fp32, isOutput=True)

    with (
        nc.sbuf_tensor([P, 2 * F], fp32) as xt,
        nc.sbuf_tensor([P, 2 * F], fp32) as et,
        nc.sbuf_tensor([P, 1], fp32) as warm,
        nc.sbuf_tensor([P, 2], fp32) as ot,
        nc.semaphore("dsem") as dsem,
        nc.semaphore("esem") as esem,
        nc.semaphore("vsem") as vsem,
        nc.Block(no_gpsimd_drain=True) as block,
    ):

        @block.sync
        def _(sync):
            sync.dma_start(out=xt[:], in_=x_d[:]).then_inc(dsem, 16)
            sync.wait_ge(vsem, 1)
            sync.dma_start(out=o_d[:], in_=ot[:]).then_inc(dsem, 16)
            sync.wait_ge(dsem, 32)  # out DMA fully landed before NEFF end

        @block.scalar
        def _(scalar):
            # Warm the Exp activation table while the input DMA is in flight.
            zero = nc.const_aps.scalar_like(0.0, warm[:, 0:1])
            scalar.activation(warm[:, 0:1], zero, mybir.ActivationFunctionType.Exp)
            scalar.wait_ge(dsem, 16)
            scalar.activation(
                et[:], xt[:], mybir.ActivationFunctionType.Exp
            ).then_inc(esem, 1)

        @block.vector
        def _(vector):
            vector.wait_ge(esem, 1)
            # [128, 2, 64] -> [128, 2]: col 0 = sum exp(w), col 1 = sum exp(v)
            vector.reduce_sum(
                ot[:, 0:2],
                et[:].rearrange("p (g f) -> p g f", g=2),
                axis=mybir.AxisListType.X,
            ).then_inc(vsem, 1)

    # Raw Bass skips Bacc's codegen_inst_isa_subclasses pass; without it any
    # extended-ISA instructions have empty .instr bytes and walrus codegen
    # fails with "ISA wrong length".
    mybir.codegen_inst_isa_subclasses(nc)
    return nc


def _get_nc():
    if "nc" not in _NC_CACHE:
        _NC_CACHE["nc"] = _build_bass()
    return _NC_CACHE["nc"]


def _pack(input, target):
    """Per-core [128, 128] f32: cols 0:64 = w, cols 64:128 = v."""
    maps = []
    for i in range(N_CORES):
        sl = slice(i * BPC, (i + 1) * BPC)
        x = input[sl]
        pos = target[sl] == 1
        buf = np.empty((P, 2 * F), dtype=np.float32)
        buf[:, :F] = np.where(pos, -BIG, x).reshape(P, F)
        buf[:, F:] = np.where(pos, -x, -BIG).reshape(P, F)
        maps.append({"x": buf})
    return maps


def kernel(input, target, _results_out=None):
    input = np.ascontiguousarray(np.asarray(input, dtype=np.float32))
    target = np.ascontiguousarray(np.asarray(target, dtype=np.int32))
    assert input.shape == (B, C) and target.shape == (B, C)

    nc = _get_nc()
    in_maps = _pack(input, target)
    res = run_bass_kernel_spmd(nc, in_maps, core_ids=list(range(N_CORES)), **_RUN_KWARGS)
    if _results_out is not None:
        _results_out.append(res)

    n_pos = target.sum(axis=1).astype(np.float32)          # [B]
    y_norm = n_pos * (np.float32(C) - n_pos)               # [B]
    total = np.float32(0.0)
    for i in range(N_CORES):
        stats = res.results[i]["out"]                      # [128, 2] f32
        per_sample = stats.reshape(BPC, PPS, 2).sum(axis=1, dtype=np.float32)
        s_neg, s_posinv = per_sample.T                     # [4], [4]
        yn = y_norm[i * BPC : (i + 1) * BPC]
        total = total + np.sum(s_posinv * s_neg / yn, dtype=np.float32)
    return np.asarray(total / np.float32(B), dtype=np.float32)


if __name__ == "__main__":
    rng = np.random.default_rng(0)
    inp = rng.standard_normal((B, C), dtype=np.float32)
    tgt = rng.integers(0, 2, size=(B, C)).astype(np.int32)
    print(kernel(input=inp, target=tgt))


# revision 16
# speedup vs baseline: 1.2232x; 1.2232x over previous
"""BP-MLL loss kernel for Trainium2 (8 NeuronCores, data-parallel over batch).

Math: for each sample b with scores o and binary labels y,
  pair_sums[b] = sum_{i in pos, j in neg} exp(o_j - o_i)
               = (sum_{j in neg} exp(o_j)) * (sum_{i in pos} exp(-o_i))
  y_norm[b]    = n_pos * (C - n_pos)
  loss         = sum_b pair_sums[b] / y_norm[b] / B

Since labels are 0/1, the masks fold into the exp arguments on the host:
  w = where(y==0,  x, -BIG)   ->  exp(w) = (1-y)*exp(x)   (underflows to 0)
  v = where(y==1, -x, -BIG)   ->  exp(v) =     y*exp(-x)
Each core gets 4 samples packed as one [128, 128] f32 buffer (cols 0:64 = w,
cols 64:128 = v; sample b owns partitions 32b..32b+31). The device runs a
single Exp activation and a single 2-group free-axis reduce, emitting
[128, 2] per-partition partial sums; the host finishes the tiny segmented
reduction (n_pos comes straight from `target` on the host).
"""

import sys

for _p in ("/opt/trn_rl_repo", "/root/.axon_site/_ro/trn_rl_repo"):
    if _p not in sys.path:
        sys.path.insert(0, _p)

import numpy as np

import concourse.bass as bass
import concourse.mybir as mybir
from concourse.bass_utils import run_bass_kernel_spmd

B, C = 32, 2048
N_CORES = 8
BPC = B // N_CORES            # samples per core (4)
P = 128                       # SBUF partitions
F = BPC * C // P              # free elems per partition (64)
PPS = P // BPC                # partitions per sample (32)
BIG = np.float32(30000.0)     # exp(-BIG) underflows to +0 (masked-out entries)

_NC_CACHE = {}
# Extra kwargs for run_bass_kernel_spmd (e.g. trace=True from a test harness).
_RUN_KWARGS = {}


def _build_bass():
    nc = bass.Bass("TRN2", enable_partition_id=False)
    fp32 = mybir.dt.float32
    x_d = nc.declare_dram_parameter("x", [P, 2 * F], fp32, isOutput=False)
    o_d = nc.declare_dram_parameter("out", [P, 2], fp32, isOutput=True)

    with (
        nc.sbuf_tensor([P, 2 * F], fp32) as xt,
        nc.sbuf_tensor([P, 2 * F], fp32) as et,
        nc.sbuf_tensor([P, 1], fp32) as warm,
        nc.sbuf_tensor([P, 2], fp32) as ot,
        nc.semaphore("dsem") as dsem,
        nc.semaphore("esem") as esem,
        nc.semaphore("vsem") as vsem,
    ):
        # Straight-line emission (no nc.Block): saves the per-engine body
        # branches, while the explicit drain + sem-only barrier below keeps
        # the exact retire semantics of nc.Block(no_gpsimd_drain=True) —
        # which is what guarantees the out DMA has quiesced before NEFF end
        # (verified with an unwaited 6MB final DMA: zero corruption).

        # Warm the Exp activation table while the input DMA is in flight.
        zero = nc.const_aps.scalar_like(0.0, warm[:, 0:1])
        nc.scalar.activation(warm[:, 0:1], zero, mybir.ActivationFunctionType.Exp)
        nc.sync.dma_start(out=xt[:], in_=x_d[:]).then_inc(dsem, 16)
        nc.scalar.wait_ge(dsem, 16)
        nc.scalar.activation(
            et[:], xt[:], mybir.ActivationFunctionType.Exp
        ).then_inc(esem, 1)
        nc.vector.wait_ge(esem, 1)
        # [128, 2, 64] -> [128, 2]: col 0 = sum exp(w), col 1 = sum exp(v)
        nc.vector.reduce_sum(
            ot[:, 0:2],
            et[:].rearrange("p (g f) -> p g f", g=2),
            axis=mybir.AxisListType.X,
        ).then_inc(vsem, 1)
        nc.sync.wait_ge(vsem, 1)
        nc.sync.dma_start(out=o_d[:], in_=ot[:]).then_inc(dsem, 16)

        # Retire barrier (drains included) over exactly the engines that did
        # work. Tensor is idle all kernel and GpSimd only ran the framework
        # const memsets (retired at the init barrier), so neither needs to
        # participate.
        nc.multi_engine_barrier([nc.sync.engine, nc.scalar.engine, nc.vector.engine])

    # Raw Bass skips Bacc's codegen_inst_isa_subclasses pass; without it any
    # extended-ISA instructions have empty .instr bytes and walrus codegen
    # fails with "ISA wrong length".
    mybir.codegen_inst_isa_subclasses(nc)
    return nc


def _get_nc():
    if "nc" not in _NC_CACHE:
        _NC_CACHE["nc"] = _build_bass()
    return _NC_CACHE["nc"]


def _pack(input, target):
    """Per-core [128, 128] f32: cols 0:64 = w, cols 64:128 = v."""
    maps = []
    for i in range(N_CORES):
        sl = slice(i * BPC, (i + 1) * BPC)
        x = input[sl]
        pos = target[sl] == 1
        buf = np.empty((P, 2 * F), dtype=np.float32)
        buf[:, :F] = np.where(pos, -BIG, x).reshape(P, F)
        buf[:, F:] = np.where(pos, -x, -BIG).reshape(P, F)
        maps.append({"x": buf})
    return maps


def kernel(input, target, _results_out=None):
    input = np.ascontiguousarray(np.asarray(input, dtype=np.float32))
    target = np.ascontiguousarray(np.asarray(target, dtype=np.int32))
    assert input.shape == (B, C) and target.shape == (B, C)

    nc = _get_nc()
    in_maps = _pack(input, target)
    res = run_bass_kernel_spmd(nc, in_maps, core_ids=list(range(N_CORES)), **_RUN_KWARGS)
    if _results_out is not None:
        _results_out.append(res)

    n_pos = target.sum(axis=1).astype(np.float32)          # [B]
    y_norm = n_pos * (np.float32(C) - n_pos)               # [B]
    total = np.float32(0.0)
    for i in range(N_CORES):
        stats = res.results[i]["out"]                      # [128, 2] f32
        per_sample = stats.reshape(BPC, PPS, 2).sum(axis=1, dtype=np.float32)
        s_neg, s_posinv = per_sample.T                     # [4], [4]
        yn = y_norm[i * BPC : (i + 1) * BPC]
        total = total + np.sum(s_posinv * s_neg / yn, dtype=np.float32)
    return np.asarray(total / np.float32(B), dtype=np.float32)


if __name__ == "__main__":
    rng = np.random.default_rng(0)
    inp = rng.standard_normal((B, C), dtype=np.float32)
    tgt = rng.integers(0, 2, size=(B, C)).astype(np.int32)
    print(kernel(input=inp, target=tgt))


# revision 17
# speedup vs baseline: 1.4299x; 1.1690x over previous
"""BP-MLL loss kernel for Trainium2 (8 NeuronCores, data-parallel over batch).

Math: for each sample b with scores o and binary labels y,
  pair_sums[b] = sum_{i in pos, j in neg} exp(o_j - o_i)
               = (sum_{j in neg} exp(o_j)) * (sum_{i in pos} exp(-o_i))
  y_norm[b]    = n_pos * (C - n_pos)
  loss         = sum_b pair_sums[b] / y_norm[b] / B

Since labels are 0/1, the masks fold into the exp arguments on the host:
  w = where(y==0,  x, -BIG)   ->  exp(w) = (1-y)*exp(x)   (underflows to 0)
  v = where(y==1, -x, -BIG)   ->  exp(v) =     y*exp(-x)
Each core gets 4 samples packed as one [128, 128] f32 buffer (cols 0:64 = w,
cols 64:128 = v; sample b owns partitions 32b..32b+31). The device runs a
single Exp activation and a single 2-group free-axis reduce, emitting
[128, 2] per-partition partial sums; the host finishes the tiny segmented
reduction (n_pos comes straight from `target` on the host).
"""

import sys

for _p in ("/opt/trn_rl_repo", "/root/.axon_site/_ro/trn_rl_repo"):
    if _p not in sys.path:
        sys.path.insert(0, _p)

import numpy as np

import concourse.bass as bass
import concourse.mybir as mybir
from concourse.bass_utils import run_bass_kernel_spmd

B, C = 32, 2048
N_CORES = 8
BPC = B // N_CORES            # samples per core (4)
P = 128                       # SBUF partitions
F = BPC * C // P              # free elems per partition (64)
PPS = P // BPC                # partitions per sample (32)
BIG = np.float32(30000.0)     # exp(-BIG) underflows to +0 (masked-out entries)

_NC_CACHE = {}
# Extra kwargs for run_bass_kernel_spmd (e.g. trace=True from a test harness).
_RUN_KWARGS = {}


def _build_bass():
    nc = bass.Bass("TRN2", enable_partition_id=False)
    # Snapshot framework init instructions (const memsets + init all-engine
    # barrier). Nothing in this kernel depends on them — the Exp bias rides
    # in the input DMA as a host-zeroed extra column — so they are deleted
    # below, pulling the input DMA issue ~1us earlier.
    pre = set()
    for f in nc.m.functions:
        for bb in f.blocks:
            for inst in bb.instructions:
                pre.add(inst.name)

    fp32 = mybir.dt.float32
    x_d = nc.declare_dram_parameter("x", [P, 2 * F + 1], fp32, isOutput=False)
    o_d = nc.declare_dram_parameter("out", [P, 2], fp32, isOutput=True)

    with (
        nc.sbuf_tensor([P, 2 * F + 1], fp32) as xt,
        nc.sbuf_tensor([P, 2 * F], fp32) as et,
        nc.sbuf_tensor([P, 1], fp32) as warm,
        nc.sbuf_tensor([P, 2], fp32) as ot,
        nc.semaphore("dsem") as dsem,
        nc.semaphore("esem") as esem,
        nc.semaphore("vsem") as vsem,
    ):
        # Straight-line emission (no nc.Block): saves the per-engine body
        # branches, while the explicit drain + sem-only barrier below keeps
        # the exact retire semantics of nc.Block(no_gpsimd_drain=True) —
        # which is what guarantees the out DMA has quiesced before NEFF end
        # (verified with an unwaited 6MB final DMA: zero corruption).

        # Warm the Exp activation table while the input DMA is in flight
        # (garbage input/bias is fine — only the table load matters).
        nc.scalar.activation(warm[:, 0:1], warm[:, 0:1],
                             mybir.ActivationFunctionType.Exp, bias=warm[:, 0:1])
        nc.sync.dma_start(out=xt[:], in_=x_d[:]).then_inc(dsem, 16)
        nc.scalar.wait_ge(dsem, 16)
        nc.scalar.activation(
            et[:], xt[:, 0 : 2 * F], mybir.ActivationFunctionType.Exp,
            bias=xt[:, 2 * F : 2 * F + 1],
        ).then_inc(esem, 1)
        nc.vector.wait_ge(esem, 1)
        # [128, 2, 64] -> [128, 2]: col 0 = sum exp(w), col 1 = sum exp(v)
        nc.vector.reduce_sum(
            ot[:, 0:2],
            et[:].rearrange("p (g f) -> p g f", g=2),
            axis=mybir.AxisListType.X,
        ).then_inc(vsem, 1)
        nc.sync.wait_ge(vsem, 1)
        nc.sync.dma_start(out=o_d[:], in_=ot[:]).then_inc(dsem, 16)

        # Retire barrier (drains included) over exactly the engines that did
        # work. Tensor is idle all kernel and GpSimd only ran the framework
        # const memsets (retired at the init barrier), so neither needs to
        # participate.
        nc.multi_engine_barrier([nc.sync.engine, nc.scalar.engine, nc.vector.engine])

    # Delete the framework init instructions (memsets/drains/evsems only —
    # structural ops like the entry dummycall must stay).
    DEL = (mybir.InstMemset, mybir.InstDrain, mybir.InstEventSemaphore)
    for f in nc.m.functions:
        for bb in f.blocks:
            keep = [i for i in bb.instructions
                    if not (i.name in pre and isinstance(i, DEL))]
            del bb.instructions[:]
            bb.instructions.extend(keep)

    # Raw Bass skips Bacc's codegen_inst_isa_subclasses pass; without it any
    # extended-ISA instructions have empty .instr bytes and walrus codegen
    # fails with "ISA wrong length".
    mybir.codegen_inst_isa_subclasses(nc)
    return nc


def _get_nc():
    if "nc" not in _NC_CACHE:
        _NC_CACHE["nc"] = _build_bass()
    return _NC_CACHE["nc"]


def _pack(input, target):
    """Per-core [128, 128] f32: cols 0:64 = w, cols 64:128 = v."""
    maps = []
    for i in range(N_CORES):
        sl = slice(i * BPC, (i + 1) * BPC)
        x = input[sl]
        pos = target[sl] == 1
        buf = np.zeros((P, 2 * F + 1), dtype=np.float32)
        buf[:, :F] = np.where(pos, -BIG, x).reshape(P, F)
        buf[:, F : 2 * F] = np.where(pos, -x, -BIG).reshape(P, F)
        maps.append({"x": buf})
    return maps


def kernel(input, target, _results_out=None):
    input = np.ascontiguousarray(np.asarray(input, dtype=np.float32))
    target = np.ascontiguousarray(np.asarray(target, dtype=np.int32))
    assert input.shape == (B, C) and target.shape == (B, C)

    nc = _get_nc()
    in_maps = _pack(input, target)
    res = run_bass_kernel_spmd(nc, in_maps, core_ids=list(range(N_CORES)), **_RUN_KWARGS)
    if _results_out is not None:
        _results_out.append(res)

    n_pos = target.sum(axis=1).astype(np.float32)          # [B]
    y_norm = n_pos * (np.float32(C) - n_pos)               # [B]
    total = np.float32(0.0)
    for i in range(N_CORES):
        stats = res.results[i]["out"]                      # [128, 2] f32
        per_sample = stats.reshape(BPC, PPS, 2).sum(axis=1, dtype=np.float32)
        s_neg, s_posinv = per_sample.T                     # [4], [4]
        yn = y_norm[i * BPC : (i + 1) * BPC]
        total = total + np.sum(s_posinv * s_neg / yn, dtype=np.float32)
    return np.asarray(total / np.float32(B), dtype=np.float32)


if __name__ == "__main__":
    rng = np.random.default_rng(0)
    inp = rng.standard_normal((B, C), dtype=np.float32)
    tgt = rng.integers(0, 2, size=(B, C)).astype(np.int32)
    print(kernel(input=inp, target=tgt))
